# revision 1
# baseline (speedup 1.0000x reference)
"""Trainium2 Bass kernel for nn_Attention_81655918231876.

RoPE attention with positional bias, 8 heads / dim_head 64, b=2, n=2048, dim=512.
Sharding: head-parallel across 8 cores. Core h computes head h for BOTH batches
and ships the per-head attention output O_h^T (bf16) plus softmax row sums
(f32); the host applies 1/rowsum, the w_out projection, and the head sum.

Design notes (all-bf16 matmul path; ~3x margin under the 2e-2 gate):
  - Projections: stationary weight blocks [q|qrot], [k|krot], [v|pad] in bf16,
    moving x^T chunks, two 512-token chunks per PSUM group; RoPE combine is
    two DVE muls against a stacked cos/sin table plus one DVE add writing the
    bf16 q/k packs ([batch0; batch1] on partitions).
  - S = q k^T as plain bf16 K=64 matmuls; the two batches' matmuls use PE
    row groups 0/64 (tile_position via base partition) and run CONCURRENTLY.
  - exp(S) on ScalarE -> bf16; bias multiply exp(S)*exp(bias) on DVE at 2x
    bf16 rate (one [128,2,512] op; the eb block is a stride-0 broadcast).
    All elementwise stays on DVE: GPSIMD shares the SBUF port with DVE and
    concurrent Pool tensor ops halve DVE throughput (measured).
  - P V accumulated with an extra ones-column in V so row 64 of O^T is the
    softmax row sum (M=65); V natural layout built by one DMA xbar transpose
    per batch (destination blocks 32B-aligned, VSTRIDE=80).
  - Software pipeline: quarter-0's S/exp/mult overlaps the projection tail
    via a filler queue; each quarter's PV + copies + DMAs drain as fillers
    inside the NEXT quarter's exp-paced loop, so the PE stream stays dense
    (HAM stays at K=8/8) and GPSIMD-free mults have a full quarter of slack.
"""

import numpy as np
import ml_dtypes
import sys

sys.path.insert(0, "/opt/trn_rl_repo")

HEADS = 8
DIM_HEAD = 64
ROPE_THETA = 10000.0
B, N, DIM = 2, 2048, 512
# per-j-block column stride in vsb: 64 V cols + 1 ones col + pad. Must keep
# every block's byte offset 32B-aligned: the DMA xbar transpose writes in
# 16-element (bf16) groups and silently corrupts unaligned destinations.
VSTRIDE = 80

_compiled = None
_DEBUG = False


def _build():
    import concourse.bass as bass
    import concourse.tile as tile
    from concourse import bacc, mybir

    f32 = mybir.dt.float32
    bf16 = mybir.dt.bfloat16
    Exp = mybir.ActivationFunctionType.Exp
    Copy = mybir.ActivationFunctionType.Copy

    nc = bacc.Bacc(None, target_bir_lowering=False, debug=False)
    xt = nc.dram_tensor("xt", [DIM, 2 * N], bf16, kind="ExternalInput")
    wall = nc.dram_tensor("wall", [DIM, 384], bf16, kind="ExternalInput")
    cs2 = nc.dram_tensor("cs2", [128, N], f32, kind="ExternalInput")
    ebt = nc.dram_tensor("ebt", [N, N], bf16, kind="ExternalInput")
    oto = nc.dram_tensor("oto", [4 * B, 64, 512], bf16, kind="ExternalOutput")
    rsum = nc.dram_tensor("rsum", [4 * B, 512], f32, kind="ExternalOutput")
    if _DEBUG:
        dbg_qkv = nc.dram_tensor("dbg_qkv", [3, 128, N], bf16, kind="ExternalOutput")
        dbg_vsb = nc.dram_tensor("dbg_vsb", [B, 128, 16 * VSTRIDE], bf16, kind="ExternalOutput")
        dbg_pt = nc.dram_tensor("dbg_pt", [128, 1024], bf16, kind="ExternalOutput")
        dbg_rb = nc.dram_tensor("dbg_rb", [64, 512], f32, kind="ExternalOutput")

    with tile.TileContext(nc) as tc:
        with (
            tc.tile_pool(name="singles", bufs=1) as singles,
            tc.tile_pool(name="t12p", bufs=2) as t12p,
            tc.tile_pool(name="ptsp", bufs=3) as ptsp,
            tc.tile_pool(name="ptp", bufs=17) as ptp,
            tc.tile_pool(name="rrp", bufs=2) as rrp,
            tc.tile_pool(name="otp", bufs=2) as otp,
            tc.tile_pool(name="ysp", bufs=2) as ysp,
            tc.tile_pool(name="psS", bufs=2, space="PSUM") as psS,
        ):
            # ---- constants / inputs ----
            wl = [singles.tile([128, 384], bf16, tag=f"wl{k}", name=f"wl{k}") for k in range(4)]
            for k in range(4):
                nc.sync.dma_start(out=wl[k], in_=wall[128 * k:128 * (k + 1), :])
            xb = [singles.tile([128, 2 * N], bf16, tag=f"xb{k}", name=f"xb{k}") for k in range(4)]
            for lo, hi in ((0, 1024), (1024, 2048), (2048, 4096)):
                for k in range(4):
                    nc.sync.dma_start(
                        out=xb[k][:, lo:hi],
                        in_=xt[128 * k:128 * (k + 1), lo:hi],
                    )
            cs_sb = singles.tile([128, N], f32, tag="cs", name="cs_sb")
            nc.sync.dma_start(out=cs_sb, in_=cs2[:, :])
            eb_sb = singles.tile([128, 16 * N], bf16, tag="eb", name="eb_sb")
            for j in range(16):
                nc.sync.dma_start(
                    out=eb_sb[:, N * j:N * (j + 1)],
                    in_=ebt[128 * j:128 * (j + 1), :],
                )

            qb = singles.tile([128, N], bf16, tag="qb", name="qb")
            kb = singles.tile([128, N], bf16, tag="kb", name="kb")
            vt = singles.tile([128, N], bf16, tag="vt", name="vt")
            vsb = [singles.tile([128, 16 * VSTRIDE], bf16, tag=f"vsb{b}", name=f"vsb{b}")
                   for b in range(B)]
            for b in range(B):
                nc.vector.memset(vsb[b], 1.0)

            # ---- pipelined projection + attention ----
            # Phase 1 (psS + psP open): dense proj of k-pairs and the first
            # q token range, then quarter-0's S/exp/mult loop with the rest
            # of the projection (q tail, v, xbar transposes) as PE/DVE/Pool
            # fillers. PV always runs one quarter behind as fillers, so the
            # GPSIMD bias-mults have a full quarter of slack.
            from collections import deque
            fillers = deque()

            def emit_fill(n):
                for _ in range(n):
                    if fillers:
                        fillers.popleft()()

            def proj_group_closures(mt, cpair, pool=None):
                out = []

                def mms(k):
                    def f():
                        tile = proj_group_closures.tiles.get((mt, cpair[0]))
                        if tile is None:
                            p = psP if pool is None else pool
                            tile = p.tile([128, 1024], f32, tag="pp",
                                          name=f"pp_{mt}_{cpair[0]}")
                            proj_group_closures.tiles[(mt, cpair[0])] = tile
                        for ci, c in enumerate(cpair):
                            nc.tensor.matmul(
                                tile[:, 512 * ci:512 * (ci + 1)],
                                wl[k][:, 128 * mt:128 * (mt + 1)],
                                xb[k][:, 512 * c:512 * (c + 1)],
                                start=(k == 0), stop=(k == 3),
                            )
                    return f
                for k in range(4):
                    out.append(mms(k))

                b = cpair[0] // 4
                tok = 512 * (cpair[0] % 4)
                if mt < 2:
                    dst = qb if mt == 0 else kb

                    def rope():
                        tile = proj_group_closures.tiles.pop((mt, cpair[0]))
                        t1 = t12p.tile([64, 1024], f32, tag="t1",
                                       name=f"t1f_{mt}_{cpair[0]}")
                        t2 = t12p.tile([64, 1024], f32, tag="t2",
                                       name=f"t2f_{mt}_{cpair[0]}")
                        nc.vector.tensor_mul(t1, tile[0:64, :],
                                             cs_sb[0:64, tok:tok + 1024])
                        nc.vector.tensor_mul(t2, tile[64:128, :],
                                             cs_sb[64:128, tok:tok + 1024])
                        nc.vector.tensor_add(
                            dst[64 * b:64 * b + 64, tok:tok + 1024], t1, t2)
                    out.append(rope)
                else:
                    def vcopy():
                        tile = proj_group_closures.tiles.pop((mt, cpair[0]))
                        nc.scalar.activation(
                            vt[64 * b:64 * b + 64, tok:tok + 1024],
                            tile[0:64, :], Copy)
                    out.append(vcopy)
                return out
            proj_group_closures.tiles = {}

            def proj_group(mt, cpair):
                """Dense variant: emit the 8 matmuls now, return the rope/v
                closures to drain while the next group's matmuls stream."""
                tile = psP.tile([128, 1024], f32, tag="pp",
                                name=f"pp_{mt}_{cpair[0]}")
                for k in range(4):
                    for ci, c in enumerate(cpair):
                        nc.tensor.matmul(
                            tile[:, 512 * ci:512 * (ci + 1)],
                            wl[k][:, 128 * mt:128 * (mt + 1)],
                            xb[k][:, 512 * c:512 * (c + 1)],
                            start=(k == 0), stop=(k == 3),
                        )
                b = cpair[0] // 4
                tok = 512 * (cpair[0] % 4)
                post = []
                if mt < 2:
                    dst = qb if mt == 0 else kb
                    t1 = t12p.tile([64, 1024], f32, tag="t1",
                                   name=f"t1_{mt}_{cpair[0]}")
                    t2 = t12p.tile([64, 1024], f32, tag="t2",
                                   name=f"t2_{mt}_{cpair[0]}")
                    post.append(lambda: nc.vector.tensor_mul(
                        t1, tile[0:64, :], cs_sb[0:64, tok:tok + 1024]))
                    post.append(lambda: nc.vector.tensor_mul(
                        t2, tile[64:128, :], cs_sb[64:128, tok:tok + 1024]))
                    post.append(lambda: nc.vector.tensor_add(
                        dst[64 * b:64 * b + 64, tok:tok + 1024], t1, t2))
                else:
                    post.append(lambda: nc.scalar.activation(
                        vt[64 * b:64 * b + 64, tok:tok + 1024],
                        tile[0:64, :], Copy))
                return post

            def vtrans(b):
                def f():
                    dst = vsb[b].rearrange("p (j c) -> p j c",
                                           c=VSTRIDE)[:, :, 0:64]
                    nc.sync.dma_start_transpose(dst, vt[64 * b:64 * b + 64, :])
                return f

            pt_store = {}

            def s_loop(q, fill_per_step):
                """16-step S/exp/mult loop for quarter q; P tiles are kept
                for the next quarter's PV fillers."""
                i0 = 512 * q
                budgets = (fill_per_step if isinstance(fill_per_step, list)
                           else [fill_per_step] * 16)
                for j in range(16):
                    s_ps = psS.tile([128, 1024], f32, tag="s",
                                    name=f"s_{q}_{j}")
                    for b in range(B):
                        nc.tensor.matmul(
                            s_ps[:, 512 * b:512 * (b + 1)],
                            kb[64 * b:64 * b + 64, 128 * j:128 * (j + 1)],
                            qb[64 * b:64 * b + 64, i0:i0 + 512],
                            start=True, stop=True,
                        )
                    pts = ptsp.tile([128, 1024], bf16, tag="pts",
                                    name=f"pts_{q}_{j}")
                    nc.scalar.activation(pts, s_ps, Exp)
                    pt = ptp.tile([128, 1024], bf16, tag="pt",
                                  name=f"pt_{q}_{j}")
                    ebs = eb_sb[:, N * j + i0:N * j + i0 + 512]
                    eng = nc.vector
                    eng.tensor_mul(
                        pt.rearrange("p (r c) -> p r c", r=2),
                        pts.rearrange("p (r c) -> p r c", r=2),
                        ebs.unsqueeze(1).broadcast_to((128, 2, 512)))
                    pt_store[(q, j)] = pt
                    emit_fill(budgets[j])

            def quarter_drain_closures(q):
                """PV + rowsum/O copies + output projection for quarter q,
                as closures to interleave into the next quarter's loop."""
                i0 = 512 * q
                ots = [psO.tile([65, 512], f32, tag=f"o{b}",
                                name=f"ot_{b}_{q}") for b in range(B)]
                out_cl = []

                def mk_pv(j):
                    def f():
                        for b in range(B):
                            nc.tensor.matmul(
                                ots[b],
                                vsb[b][:, VSTRIDE * j:VSTRIDE * j + 65],
                                pt_store.pop((q, j))[:, 512 * b:512 * (b + 1)]
                                if b == B - 1 else
                                pt_store[(q, j)][:, 512 * b:512 * (b + 1)],
                                start=(j == 0), stop=(j == 15),
                            )
                    return f
                out_cl += [mk_pv(j) for j in range(16)]

                def mk_copies(b):
                    ot = ots[b]
                    rs = rrp.tile([1, 512], f32, tag="rs", name=f"rs_{b}_{q}")
                    otsb = otp.tile([64, 512], bf16, tag=f"otsb{b}",
                                    name=f"otsb_{b}_{q}")

                    def f():
                        nc.vector.tensor_copy(rs, ot[64:65, :])
                        nc.sync.dma_start(
                            out=rsum[4 * b + q:4 * b + q + 1, :], in_=rs)
                        nc.vector.tensor_copy(otsb, ot[0:64, :])
                        nc.sync.dma_start(out=oto[4 * b + q, :, :], in_=otsb)
                    return f
                out_cl += [mk_copies(b) for b in range(B)]
                return out_cl

            # ---- phase 1 ----
            # Dense prelude covers only what quarter 0 needs up front
            # (k and q for the first token ranges); everything else flows
            # in as fillers inside the quarter loops.
            with tc.tile_pool(name="psP", bufs=2, space="PSUM") as psP:
                dense = [(1, (0, 1)), (1, (4, 5)), (0, (0, 1)), (0, (4, 5))]
                pending = []
                for mt, cpair in dense:
                    for p in pending:
                        p()
                    pending = proj_group(mt, cpair)
                for p in pending:
                    p()
                for mt, cpair in ((1, (2, 3)), (1, (6, 7)), (2, (0, 1)),
                                  (2, (2, 3))):
                    fillers.extend(proj_group_closures(mt, cpair))
                fillers.append(vtrans(0))
                for mt, cpair in ((2, (4, 5)), (2, (6, 7))):
                    fillers.extend(proj_group_closures(mt, cpair))
                fillers.append(vtrans(1))
                # pre-pump filler matmuls: S(0,0) head-of-line blocks the
                # in-order PE queue on the DVE rope chain; these k/v matmuls
                # only need resident x/weights and fill that window
                emit_fill(11)
                s_loop(0, 3)
                emit_fill(len(fillers))

            if _DEBUG:
                nc.sync.dma_start(out=dbg_qkv[0, :, :], in_=qb)
                nc.sync.dma_start(out=dbg_qkv[1, :, :], in_=kb)
                nc.sync.dma_start(out=dbg_qkv[2, :, :], in_=vt)
                for b in range(B):
                    nc.sync.dma_start(out=dbg_vsb[b, :, :], in_=vsb[b])

            # ---- phase 2 ----
            with (
                tc.tile_pool(name="psO", bufs=1, space="PSUM") as psO,
                tc.tile_pool(name="psQ", bufs=1, space="PSUM") as psQ,
            ):
                for mt, cpair in ((0, (2, 3)), (0, (6, 7))):
                    fillers.extend(proj_group_closures(mt, cpair, pool=psQ))
                fillers.extend(quarter_drain_closures(0))
                s_loop(1, 3)
                fillers.extend(quarter_drain_closures(1))
                s_loop(2, 2)
                fillers.extend(quarter_drain_closures(2))
                fillers.extend(quarter_drain_closures(3))
                s_loop(3, 2)
                emit_fill(len(fillers))

    nc.compile()
    return nc


def _host_inputs(x, pos_bias, w_qkv, w_out):
    """Build the per-core input maps (head-parallel sharding)."""
    bf = ml_dtypes.bfloat16
    x = np.asarray(x, dtype=np.float32)
    pos_bias = np.asarray(pos_bias, dtype=np.float32)
    w_qkv = np.asarray(w_qkv, dtype=np.float32)
    w_out = np.asarray(w_out, dtype=np.float32)
    hidden = HEADS * DIM_HEAD

    xt = np.ascontiguousarray(
        np.concatenate([x[0].T, x[1].T], axis=1)).astype(bf)  # [512, 4096]

    inv_freq = 1.0 / (ROPE_THETA ** (np.arange(0, DIM_HEAD, 2, dtype=np.float64) / DIM_HEAD))
    freqs = np.arange(N, dtype=np.float64)[:, None] * inv_freq[None, :]
    freqs = np.repeat(freqs, 2, axis=-1)  # [n, 64]
    cosT = np.cos(freqs).T.astype(np.float32)
    sinT = np.sin(freqs).T.astype(np.float32)
    cs2 = np.ascontiguousarray(np.concatenate([cosT, sinT], axis=0))  # [128, n]

    def rot_cols(w):
        wr = np.empty_like(w)
        wr[:, 0::2] = -w[:, 1::2]
        wr[:, 1::2] = w[:, 0::2]
        return wr

    scale = DIM_HEAD ** -0.5
    in_maps = []
    for h in range(HEADS):
        wq = w_qkv[:, h * 64:(h + 1) * 64] * scale
        wk = w_qkv[:, hidden + h * 64:hidden + (h + 1) * 64]
        wvh = w_qkv[:, 2 * hidden + h * 64:2 * hidden + (h + 1) * 64]
        wall = np.ascontiguousarray(
            np.concatenate(
                [wq, rot_cols(wq), wk, rot_cols(wk), wvh,
                 np.zeros((DIM, 64), dtype=np.float32)], axis=1)
        ).astype(bf)  # [512, 384]
        in_maps.append({
            "xt": xt,
            "wall": wall,
            "cs2": cs2,
            "ebt": np.ascontiguousarray(np.exp(pos_bias[h]).T).astype(bf),
        })
    return in_maps


def kernel(x, pos_bias, w_qkv, w_out, _want_trace=False):
    global _compiled
    from concourse.bass_utils import run_bass_kernel_spmd

    if _compiled is None:
        _compiled = _build()
    in_maps = _host_inputs(x, pos_bias, w_qkv, w_out)
    res = run_bass_kernel_spmd(
        _compiled, in_maps, core_ids=list(range(HEADS)), trace=_want_trace
    )
    w_out = np.asarray(w_out, dtype=np.float32)
    y = np.zeros((B, N, DIM), dtype=np.float32)
    for h, r in enumerate(res.results):
        rs = np.asarray(r["rsum"]).reshape(B, N)
        # oto: [4b+q, 64 d, 512 tok] -> O [B, N, 64]
        ot = np.asarray(r["oto"]).astype(np.float32)
        O = ot.reshape(B, 4, 64, 512).transpose(0, 1, 3, 2).reshape(B, N, 64)
        y += (O / rs[:, :, None]) @ w_out[h * 64:(h + 1) * 64, :]
    if _want_trace:
        kernel._last_results = res
    return y



# revision 3
# speedup vs baseline: 1.2564x; 1.2564x over previous
"""Trainium2 Bass kernel for nn_Attention_81655918231876.

RoPE attention with positional bias, 8 heads / dim_head 64, b=2, n=2048, dim=512.
Sharding: head-parallel across 8 cores. Core h computes head h for BOTH batches
and ships the per-head attention output O_h^T (bf16) plus softmax row sums
(f32); the host applies 1/rowsum, the w_out projection, and the head sum.

Design notes (all-bf16 matmul path; margin under the 2e-2 gate):
  - Steady state is exp-paced: Scalar does 64 x exp([128,1024]) ~ 70-80us and
    every other engine hides under it. The schedule's job is (a) first exp
    fires as early as possible, (b) Scalar never starves, (c) short tail.
  - DMA priority order: wall, x cols the dense prelude needs (b0/b1 key+q
    chunks), cos/sin table, eb j-blocks 0-3, rest of x, eb 4-15. The 8MB eb
    table must not starve the 2MB of x the first projections need.
  - Projections: stationary weight blocks [q|qrot], [k|krot], [v|pad] in bf16,
    moving x^T chunks, two 512-token chunks per PSUM group; RoPE combine is
    ONE fused DVE mul [128,1024] against the stacked cos/sin table (bf16 out)
    plus one bf16 add at 2x rate.
  - S = q k^T as plain bf16 K=64 matmuls; the two batches' matmuls use PE
    row groups 0/64 (tile_position via base partition) and run CONCURRENTLY.
  - exp(S) on ScalarE -> bf16; bias multiply exp(S)*exp(bias) on DVE at 2x
    bf16 rate (one [128,2,512] op; the eb block is a stride-0 broadcast).
  - P V accumulated with an extra ones-column in V so row 64 of O^T is the
    softmax row sum (M=65); V natural layout built by one DMA xbar transpose
    per batch (destination blocks 32B-aligned, VSTRIDE=80).
  - Software pipeline: minimal dense prelude (k + quarter-0 q projections
    only), then quarter-0's S/exp/mult overlaps the projection tail via a
    filler queue; each quarter's PV + copies + DMAs drain as fillers inside
    the NEXT quarter's exp-paced loop, so the PE stream stays dense.
"""

import numpy as np
import ml_dtypes
import sys

sys.path.insert(0, "/opt/trn_rl_repo")

HEADS = 8
DIM_HEAD = 64
ROPE_THETA = 10000.0
B, N, DIM = 2, 2048, 512
# per-j-block column stride in vsb: 64 V cols + 1 ones col + pad. Must keep
# every block's byte offset 32B-aligned: the DMA xbar transpose writes in
# 16-element (bf16) groups and silently corrupts unaligned destinations.
VSTRIDE = 80

_compiled = None
_DEBUG = False


def _build():
    import concourse.bass as bass
    import concourse.tile as tile
    from concourse import bacc, mybir

    f32 = mybir.dt.float32
    bf16 = mybir.dt.bfloat16
    Exp = mybir.ActivationFunctionType.Exp

    nc = bacc.Bacc(None, target_bir_lowering=False, debug=False)
    xt = nc.dram_tensor("xt", [DIM, 2 * N], bf16, kind="ExternalInput")
    wall = nc.dram_tensor("wall", [DIM, 384], bf16, kind="ExternalInput")
    cs2 = nc.dram_tensor("cs2", [128, N], f32, kind="ExternalInput")
    ebt = nc.dram_tensor("ebt", [N, N], bf16, kind="ExternalInput")
    oto = nc.dram_tensor("oto", [4 * B, 64, 512], bf16, kind="ExternalOutput")
    rsum = nc.dram_tensor("rsum", [4 * B, 512], f32, kind="ExternalOutput")
    if _DEBUG:
        dbg_qkv = nc.dram_tensor("dbg_qkv", [3, 128, N], bf16, kind="ExternalOutput")
        dbg_vsb = nc.dram_tensor("dbg_vsb", [B, 128, 16 * VSTRIDE], bf16, kind="ExternalOutput")

    with tile.TileContext(nc) as tc:
        with (
            tc.tile_pool(name="singles", bufs=1) as singles,
            tc.tile_pool(name="t12p", bufs=2) as t12p,
            tc.tile_pool(name="ptsp", bufs=4) as ptsp,
            tc.tile_pool(name="ptp", bufs=17) as ptp,
            tc.tile_pool(name="rrp", bufs=2) as rrp,
            tc.tile_pool(name="otp", bufs=2) as otp,
            tc.tile_pool(name="psS", bufs=2, space="PSUM") as psS,
        ):
            # ---- inputs, in bandwidth-priority order ----
            # The dense prelude needs: wall, x cols 0-1023 (b0 keys+q0) and
            # 2048-3071 (b1 keys+q0), cos/sin. eb j-blocks 0-3 must land by
            # the time quarter-0's mult loop starts (~10us); the rest of eb
            # follows the remaining x chunks.
            wl = [singles.tile([128, 384], bf16, tag=f"wl{k}", name=f"wl{k}") for k in range(4)]
            for k in range(4):
                nc.sync.dma_start(out=wl[k], in_=wall[128 * k:128 * (k + 1), :])
            xb = [singles.tile([128, 2 * N], bf16, tag=f"xb{k}", name=f"xb{k}") for k in range(4)]
            for lo, hi in ((0, 1024), (2048, 3072)):
                for k in range(4):
                    nc.sync.dma_start(
                        out=xb[k][:, lo:hi],
                        in_=xt[128 * k:128 * (k + 1), lo:hi],
                    )
            cs_sb = singles.tile([128, N], f32, tag="cs", name="cs_sb")
            nc.sync.dma_start(out=cs_sb, in_=cs2[:, :])
            eb_sb = singles.tile([128, 16 * N], bf16, tag="eb", name="eb_sb")
            for j in range(4):
                nc.sync.dma_start(
                    out=eb_sb[:, N * j:N * (j + 1)],
                    in_=ebt[128 * j:128 * (j + 1), :],
                )
            for lo, hi in ((1024, 2048), (3072, 4096)):
                for k in range(4):
                    nc.sync.dma_start(
                        out=xb[k][:, lo:hi],
                        in_=xt[128 * k:128 * (k + 1), lo:hi],
                    )
            for j in range(4, 16):
                nc.sync.dma_start(
                    out=eb_sb[:, N * j:N * (j + 1)],
                    in_=ebt[128 * j:128 * (j + 1), :],
                )

            qb = singles.tile([128, N], bf16, tag="qb", name="qb")
            kb = singles.tile([128, N], bf16, tag="kb", name="kb")
            vt = singles.tile([128, N], bf16, tag="vt", name="vt")
            vsb = [singles.tile([128, 16 * VSTRIDE], bf16, tag=f"vsb{b}", name=f"vsb{b}")
                   for b in range(B)]
            for b in range(B):
                nc.vector.memset(vsb[b], 1.0)

            # ---- pipelined projection + attention ----
            from collections import deque
            fillers = deque()

            def emit_fill(n):
                for _ in range(n):
                    if fillers:
                        fillers.popleft()()

            def rope_ops(mt, cpair, ptile):
                """RoPE combine: two muls against the stacked cos/sin table
                writing bf16 (so the add runs at 2x DVE rate). Both t1/t2
                sit at base partition 0 (TensorTensor needs equal SBUF input
                base partitions)."""
                b = cpair[0] // 4
                tok = 512 * (cpair[0] % 4)
                dst = qb if mt == 0 else kb
                t1 = t12p.tile([64, 1024], bf16, tag="t1",
                               name=f"t1_{mt}_{cpair[0]}")
                t2 = t12p.tile([64, 1024], bf16, tag="t2",
                               name=f"t2_{mt}_{cpair[0]}")
                nc.vector.tensor_mul(t1, ptile[0:64, :],
                                     cs_sb[0:64, tok:tok + 1024])
                nc.vector.tensor_mul(t2, ptile[64:128, :],
                                     cs_sb[64:128, tok:tok + 1024])
                nc.vector.tensor_add(
                    dst[64 * b:64 * b + 64, tok:tok + 1024], t1, t2)

            def vcopy_op(cpair, ptile):
                b = cpair[0] // 4
                tok = 512 * (cpair[0] % 4)
                nc.vector.tensor_copy(
                    vt[64 * b:64 * b + 64, tok:tok + 1024], ptile[0:64, :])

            def proj_group_closures(mt, cpair, pool=None):
                out = []

                def mms(k):
                    def f():
                        tile = proj_group_closures.tiles.get((mt, cpair[0]))
                        if tile is None:
                            p = psP if pool is None else pool
                            tile = p.tile([128, 1024], f32, tag="pp",
                                          name=f"pp_{mt}_{cpair[0]}")
                            proj_group_closures.tiles[(mt, cpair[0])] = tile
                        for ci, c in enumerate(cpair):
                            nc.tensor.matmul(
                                tile[:, 512 * ci:512 * (ci + 1)],
                                wl[k][:, 128 * mt:128 * (mt + 1)],
                                xb[k][:, 512 * c:512 * (c + 1)],
                                start=(k == 0), stop=(k == 3),
                            )
                    return f
                for k in range(4):
                    out.append(mms(k))

                if mt < 2:
                    def rope():
                        tile = proj_group_closures.tiles.pop((mt, cpair[0]))
                        rope_ops(mt, cpair, tile)
                    out.append(rope)
                else:
                    def vcopy():
                        tile = proj_group_closures.tiles.pop((mt, cpair[0]))
                        vcopy_op(cpair, tile)
                    out.append(vcopy)
                return out
            proj_group_closures.tiles = {}

            def proj_group(mt, cpair):
                """Dense variant: emit the 8 matmuls now, return the rope/v
                closures to drain while the next group's matmuls stream."""
                tile = psP.tile([128, 1024], f32, tag="pp",
                                name=f"pp_{mt}_{cpair[0]}")
                for k in range(4):
                    for ci, c in enumerate(cpair):
                        nc.tensor.matmul(
                            tile[:, 512 * ci:512 * (ci + 1)],
                            wl[k][:, 128 * mt:128 * (mt + 1)],
                            xb[k][:, 512 * c:512 * (c + 1)],
                            start=(k == 0), stop=(k == 3),
                        )
                if mt < 2:
                    return [lambda: rope_ops(mt, cpair, tile)]
                return [lambda: vcopy_op(cpair, tile)]

            def vtrans(b):
                def f():
                    dst = vsb[b].rearrange("p (j c) -> p j c",
                                           c=VSTRIDE)[:, :, 0:64]
                    nc.sync.dma_start_transpose(dst, vt[64 * b:64 * b + 64, :])
                return f

            pt_store = {}

            def s_loop(q, fill_per_step):
                """16-step S/exp/mult loop for quarter q; P tiles are kept
                for the next quarter's PV fillers."""
                i0 = 512 * q
                budgets = (fill_per_step if isinstance(fill_per_step, list)
                           else [fill_per_step] * 16)
                for j in range(16):
                    s_ps = psS.tile([128, 1024], f32, tag="s",
                                    name=f"s_{q}_{j}")
                    for b in range(B):
                        nc.tensor.matmul(
                            s_ps[:, 512 * b:512 * (b + 1)],
                            kb[64 * b:64 * b + 64, 128 * j:128 * (j + 1)],
                            qb[64 * b:64 * b + 64, i0:i0 + 512],
                            start=True, stop=True,
                        )
                    pts = ptsp.tile([128, 1024], bf16, tag="pts",
                                    name=f"pts_{q}_{j}")
                    nc.scalar.activation(pts, s_ps, Exp)
                    pt = ptp.tile([128, 1024], bf16, tag="pt",
                                  name=f"pt_{q}_{j}")
                    ebs = eb_sb[:, N * j + i0:N * j + i0 + 512]
                    nc.vector.tensor_mul(
                        pt.rearrange("p (r c) -> p r c", r=2),
                        pts.rearrange("p (r c) -> p r c", r=2),
                        ebs.unsqueeze(1).broadcast_to((128, 2, 512)))
                    pt_store[(q, j)] = pt
                    emit_fill(budgets[j])

            def quarter_drain_closures(q):
                """PV + rowsum/O copies + output DMA for quarter q,
                as closures to interleave into the next quarter's loop."""
                ots = [psO.tile([65, 512], f32, tag=f"o{b}",
                                name=f"ot_{b}_{q}") for b in range(B)]
                out_cl = []

                def mk_pv(j):
                    def f():
                        for b in range(B):
                            nc.tensor.matmul(
                                ots[b],
                                vsb[b][:, VSTRIDE * j:VSTRIDE * j + 65],
                                pt_store.pop((q, j))[:, 512 * b:512 * (b + 1)]
                                if b == B - 1 else
                                pt_store[(q, j)][:, 512 * b:512 * (b + 1)],
                                start=(j == 0), stop=(j == 15),
                            )
                    return f
                out_cl += [mk_pv(j) for j in range(16)]

                def mk_copies(b):
                    ot = ots[b]
                    rs = rrp.tile([1, 512], f32, tag="rs", name=f"rs_{b}_{q}")
                    otsb = otp.tile([64, 512], bf16, tag=f"otsb{b}",
                                    name=f"otsb_{b}_{q}")

                    def f():
                        nc.vector.tensor_copy(rs, ot[64:65, :])
                        nc.sync.dma_start(
                            out=rsum[4 * b + q:4 * b + q + 1, :], in_=rs)
                        nc.vector.tensor_copy(otsb, ot[0:64, :])
                        nc.sync.dma_start(out=oto[4 * b + q, :, :], in_=otsb)
                    return f
                out_cl += [mk_copies(b) for b in range(B)]
                return out_cl

            # ---- phase 1 ----
            # Dense prelude covers only what quarter 0 needs up front
            # (k and q for the first token ranges); everything else flows
            # in as fillers inside the quarter loops.
            with tc.tile_pool(name="psP", bufs=2, space="PSUM") as psP:
                dense = [(1, (0, 1)), (1, (4, 5)), (0, (0, 1)), (0, (4, 5))]
                pending = []
                for mt, cpair in dense:
                    for p in pending:
                        p()
                    pending = proj_group(mt, cpair)
                for p in pending:
                    p()
                for mt, cpair in ((1, (2, 3)), (1, (6, 7)), (2, (0, 1)),
                                  (2, (2, 3))):
                    fillers.extend(proj_group_closures(mt, cpair))
                fillers.append(vtrans(0))
                for mt, cpair in ((2, (4, 5)), (2, (6, 7))):
                    fillers.extend(proj_group_closures(mt, cpair))
                fillers.append(vtrans(1))
                # pre-pump a few filler matmuls: S(0,0) head-of-line blocks
                # the in-order PE queue on the DVE rope chain; these k/v
                # matmuls only need resident x/weights and fill that window
                emit_fill(4)
                s_loop(0, 3)
                emit_fill(len(fillers))

            if _DEBUG:
                nc.sync.dma_start(out=dbg_qkv[0, :, :], in_=qb)
                nc.sync.dma_start(out=dbg_qkv[1, :, :], in_=kb)
                nc.sync.dma_start(out=dbg_qkv[2, :, :], in_=vt)
                for b in range(B):
                    nc.sync.dma_start(out=dbg_vsb[b, :, :], in_=vsb[b])

            # ---- phase 2 ----
            with (
                tc.tile_pool(name="psO", bufs=1, space="PSUM") as psO,
                tc.tile_pool(name="psQ", bufs=1, space="PSUM") as psQ,
            ):
                for mt, cpair in ((0, (2, 3)), (0, (6, 7))):
                    fillers.extend(proj_group_closures(mt, cpair, pool=psQ))
                fillers.extend(quarter_drain_closures(0))
                s_loop(1, 3)
                fillers.extend(quarter_drain_closures(1))
                s_loop(2, 2)
                fillers.extend(quarter_drain_closures(2))
                fillers.extend(quarter_drain_closures(3))
                s_loop(3, 2)
                emit_fill(len(fillers))

    nc.compile()
    return nc


def _host_inputs(x, pos_bias, w_qkv, w_out):
    """Build the per-core input maps (head-parallel sharding)."""
    bf = ml_dtypes.bfloat16
    x = np.asarray(x, dtype=np.float32)
    pos_bias = np.asarray(pos_bias, dtype=np.float32)
    w_qkv = np.asarray(w_qkv, dtype=np.float32)
    w_out = np.asarray(w_out, dtype=np.float32)
    hidden = HEADS * DIM_HEAD

    xt = np.ascontiguousarray(
        np.concatenate([x[0].T, x[1].T], axis=1)).astype(bf)  # [512, 4096]

    inv_freq = 1.0 / (ROPE_THETA ** (np.arange(0, DIM_HEAD, 2, dtype=np.float64) / DIM_HEAD))
    freqs = np.arange(N, dtype=np.float64)[:, None] * inv_freq[None, :]
    freqs = np.repeat(freqs, 2, axis=-1)  # [n, 64]
    cosT = np.cos(freqs).T.astype(np.float32)
    sinT = np.sin(freqs).T.astype(np.float32)
    cs2 = np.ascontiguousarray(np.concatenate([cosT, sinT], axis=0))  # [128, n]

    def rot_cols(w):
        wr = np.empty_like(w)
        wr[:, 0::2] = -w[:, 1::2]
        wr[:, 1::2] = w[:, 0::2]
        return wr

    scale = DIM_HEAD ** -0.5
    in_maps = []
    for h in range(HEADS):
        wq = w_qkv[:, h * 64:(h + 1) * 64] * scale
        wk = w_qkv[:, hidden + h * 64:hidden + (h + 1) * 64]
        wvh = w_qkv[:, 2 * hidden + h * 64:2 * hidden + (h + 1) * 64]
        wall = np.ascontiguousarray(
            np.concatenate(
                [wq, rot_cols(wq), wk, rot_cols(wk), wvh,
                 np.zeros((DIM, 64), dtype=np.float32)], axis=1)
        ).astype(bf)  # [512, 384]
        in_maps.append({
            "xt": xt,
            "wall": wall,
            "cs2": cs2,
            "ebt": np.ascontiguousarray(np.exp(pos_bias[h]).T).astype(bf),
        })
    return in_maps


def kernel(x, pos_bias, w_qkv, w_out, _want_trace=False):
    global _compiled
    from concourse.bass_utils import run_bass_kernel_spmd

    if _compiled is None:
        _compiled = _build()
    in_maps = _host_inputs(x, pos_bias, w_qkv, w_out)
    res = run_bass_kernel_spmd(
        _compiled, in_maps, core_ids=list(range(HEADS)), trace=_want_trace
    )
    w_out = np.asarray(w_out, dtype=np.float32)
    y = np.zeros((B, N, DIM), dtype=np.float32)
    for h, r in enumerate(res.results):
        rs = np.asarray(r["rsum"]).reshape(B, N)
        # oto: [4b+q, 64 d, 512 tok] -> O [B, N, 64]
        ot = np.asarray(r["oto"]).astype(np.float32)
        O = ot.reshape(B, 4, 64, 512).transpose(0, 1, 3, 2).reshape(B, N, 64)
        y += (O / rs[:, :, None]) @ w_out[h * 64:(h + 1) * 64, :]
    if _want_trace:
        kernel._last_results = res
    return y
